# revision 2
# baseline (speedup 1.0000x reference)
"""DGCN layer (message passing GNN) on 8 Trainium2 NeuronCores via Bass/Tile.

Strategy (dst-sharded):
  - Nodes are bin-packed across 8 cores x 49 windows of 128 dst slots with
    BIMODAL per-window capacities (most windows packed to an exact tile
    multiple, per-window gathered tile counts ntl/nth shared across cores),
    so ceil-padding of the edge tiles is ~1% instead of ~7%; each core owns
    every edge whose dst lands in its windows, so the dst segment-sum is
    core-local.
  - The gathered feature table is feat = h * outdeg^-0.5 in bf16 (256B rows),
    replicated per core; per-edge rows fetched by dma_gather (SWDGE, 4
    queues, single_packet per-engine descriptor coalescing, addresses sorted
    ascending within each gather for HBM locality, even chunk sizes so all
    gathers have uniform duration and pack the queues smoothly).
  - The per-edge scatter weights sel[e, d] = alpha^dist (exact powers of two
    in fp8e4m3) are precomputed on host as dense [128, 128]-tile blocks and
    streamed via HWDGE as matmul rhs — no DVE work in the main loop (v1
    built them with is_equal/mult on DVE, which was the 500us bottleneck).
  - Phase-1 matmuls are bf16 lhsT x fp8 rhs with fp32 PSUM accumulation;
    phase-2 is bf16 with FWL weight loads (v1's fp32 matmuls cost ~350ns
    each vs ~128ns here).
  - Per-node output scale s_v = indeg[v]^-3/2 applied after the W matmul;
    output streamed back in bf16 and upcast on host.

Device pipeline per core, per 128-dst window (phase 2 inline per window):
  dma_gathers fetch the window's feat[src] rows (lo/hi int16-index tables);
  HWDGE streams the window's sel block; psum[f, d] += matmul(lhsT=G_tile,
  rhs=sel_tile) over the window's tiles; ACT copies psum -> agg^T (bf16);
  rst = matmul(lhsT=agg^T block, rhs=W) * s_v + bias -> DMA out.
"""

import math

import numpy as np

P = 128
ALPHA = 0.5
N_CORES = 8
SPLIT = 32768  # int16 index limit for dma_gather
GCH = 8  # tiles per dma_gather (64 descs/engine = single_packet ceiling)
N_QUEUES = 4


def _wrap_idx16(flat):
    """dma_gather index layout: entry k -> partition k%16, column k//16,
    replicated across the 8 gpsimd core groups (partitions 16-127)."""
    n = flat.shape[-1]
    assert n % 16 == 0
    cols = n // 16
    w = np.asarray(flat, np.int16).reshape(cols, 16).T  # [16, cols]
    return np.tile(w, (8, 1))  # [128, cols]


def _prep_host(h, src, dst, distance, n_cores):
    """Shard edges by dst range; build per-core padded tile arrays."""
    N, D = h.shape
    E = src.shape[0]
    npc = N // n_cores
    n_windows = (npc + P - 1) // P

    src = np.asarray(src).astype(np.int64)
    dst = np.asarray(dst).astype(np.int64)
    distance = np.asarray(distance)

    out_deg = np.bincount(src, minlength=N).astype(np.float64)
    in_deg = np.bincount(dst, minlength=N).astype(np.float64)
    s_all = in_deg**-1.5  # applied after the W matmul

    # Balanced node -> (core, window, slot) assignment with BIMODAL window
    # capacities: most windows are packed to an exact tile multiple so the
    # per-window gathered tile counts (ntl/nth, shared across cores by the
    # SPMD program) stay low — converting ceil-padding into ~1% waste.
    # Deal nodes (sorted by total degree) in rounds; within a round the
    # heaviest hi-degree nodes go to the feasible bin with the lightest
    # hi load. The host un-permutes output rows at the end.
    n_bins = n_cores * n_windows
    lo_deg = np.bincount(dst[src < SPLIT], minlength=N).astype(np.int64)
    hi_deg = np.bincount(dst[src >= SPLIT], minlength=N).astype(np.int64)
    # per-window tile budgets: enough total capacity per core plus margin
    avg_lo = lo_deg.sum() / n_bins
    avg_hi = hi_deg.sum() / n_bins
    t_lo_hi = int(math.ceil(avg_lo / P))  # upper tile count (e.g. 11)
    t_hi_hi = int(math.ceil(avg_hi / P))
    need_lo = int(avg_lo * n_windows)
    need_hi = int(avg_hi * n_windows)
    k_lo = min(
        n_windows,
        max(0, int(math.ceil((need_lo - n_windows * (t_lo_hi - 1) * P) / P)) + 6),
    )
    k_hi = min(
        n_windows,
        max(0, int(math.ceil((need_hi - n_windows * (t_hi_hi - 1) * P) / P)) + 6),
    )
    cap_lo_w = np.full(n_windows, (t_lo_hi - 1) * P, np.int64)
    cap_lo_w[:k_lo] = t_lo_hi * P
    cap_hi_w = np.full(n_windows, (t_hi_hi - 1) * P, np.int64)
    # big-hi windows at the end so big-lo and big-hi windows differ
    cap_hi_w[n_windows - k_hi :] = t_hi_hi * P
    cap_lo = np.tile(cap_lo_w, n_cores)
    cap_hi = np.tile(cap_hi_w, n_cores)

    order_nodes = np.argsort(-(lo_deg + hi_deg), kind="stable")
    node_bin = np.empty(N, np.int64)
    node_slot = np.empty(N, np.int64)
    lo_sum = np.zeros(n_bins, np.int64)
    hi_sum = np.zeros(n_bins, np.int64)
    fill = np.zeros(n_bins, np.int64)
    pos = 0
    rnd = 0
    while pos < N:
        take = min(n_bins, N - pos)
        nodes_r = order_nodes[pos : pos + take]
        # alternate rounds: heaviest-hi nodes to most-remaining-hi bins,
        # then heaviest-lo to most-remaining-lo — drives every bin toward
        # (not past) its bimodal cap in both buckets
        if rnd % 2 == 0:
            nodes_r = nodes_r[np.argsort(-hi_deg[nodes_r], kind="stable")]
            score = (cap_hi - hi_sum) * 4096 + (cap_lo - lo_sum)
        else:
            nodes_r = nodes_r[np.argsort(-lo_deg[nodes_r], kind="stable")]
            score = (cap_lo - lo_sum) * 4096 + (cap_hi - hi_sum)
        bins_r = np.argsort(-score, kind="stable")[:take]
        node_bin[nodes_r] = bins_r
        node_slot[nodes_r] = fill[bins_r]
        fill[bins_r] += 1
        lo_sum[bins_r] += lo_deg[nodes_r]
        hi_sum[bins_r] += hi_deg[nodes_r]
        pos += take
        rnd += 1
    node_core = node_bin // n_windows
    node_window = node_bin % n_windows

    core_of = node_core[dst]
    w_of = node_window[dst]
    r_of = node_slot[dst]
    is_hi = (src >= SPLIT).astype(np.int64)

    # sort edges by (core, window, lo/hi, src) — src-sorted within each
    # bucket so gather descriptors go in ascending HBM address order
    gw = (core_of * n_windows + w_of) * 2 + is_hi
    n_gw = n_cores * n_windows * 2
    counts = np.bincount(gw, minlength=n_gw)
    cl = counts.reshape(n_cores, n_windows, 2)
    # per-window gathered tile counts (max over cores — SPMD program is
    # shared); the bimodal capacities above keep most windows a tile lower
    # than the global max
    wmax = cl.max(axis=0)  # [n_windows, 2]
    ntl = np.maximum((wmax[:, 0] + P - 1) // P, 1).astype(np.int64)
    nth = np.maximum((wmax[:, 1] + P - 1) // P, 1).astype(np.int64)
    T_lo = int(ntl.max())
    T_hi = int(nth.max())
    T = T_lo + T_hi
    n_cols = n_windows * T  # static idx layout; sel is packed separately
    nlv = ntl + nth
    off = np.concatenate([[0], np.cumsum(nlv)])  # packed sel tile offsets
    n_cols_sel = int(off[-1])

    order = np.lexsort((src, gw))
    sgw = gw[order]
    win_start = np.concatenate([[0], np.cumsum(counts)[:-1]])
    q = np.arange(E, dtype=np.int64) - win_start[sgw]  # pos within group

    core_arr = sgw // (2 * n_windows)
    hi_arr = sgw % 2
    j_arr = q // P + hi_arr * T_lo  # hi tiles come after the lo tiles
    p_arr = q % P
    w_arr = (sgw // 2) % n_windows
    col_arr = w_arr * T + j_arr

    # dense sel blocks, PACKED: window w's live tiles are its ntl[w] lo
    # tiles then nth[w] hi tiles at column offset off[w]
    wvals = (np.float32(ALPHA) ** distance[order].astype(np.float32)).astype(
        np.float32
    )
    d_arr = r_of[order]
    cols_arr = off[w_arr] + q // P + hi_arr * ntl[w_arr]
    sel = np.zeros((n_cores, P, n_cols_sel * P), np.float32)
    sel[core_arr, p_arr, cols_arr * P + d_arr] = wvals

    # int16 gather indices, padded with 0 (sel 0 nullifies), table-relative
    srcrel = np.zeros((n_cores, P, n_cols), np.int64)
    srcrel[core_arr, p_arr, col_arr] = src[order] - (src[order] >= SPLIT) * SPLIT

    # wrapped idx16: per core, per window: lo block then hi block.
    # Blocks start at 64B-aligned column offsets (32 int16 cols).
    CL, CH = T_lo * 8, T_hi * 8  # int16 cols per window per table
    CLa = (CL + 31) // 32 * 32
    CHa = (CH + 31) // 32 * 32
    idx16 = np.zeros((n_cores, P, n_windows * (CLa + CHa)), np.int16)
    for c in range(n_cores):
        flat = srcrel[c].T  # [n_cols, P]: (tile, lane)
        for w in range(n_windows):
            lo = flat[w * T : w * T + T_lo].reshape(-1)
            hi = flat[w * T + T_lo : (w + 1) * T].reshape(-1)
            base = w * (CLa + CHa)
            idx16[c, :, base : base + CL] = _wrap_idx16(lo)
            idx16[c, :, base + CLa : base + CLa + CH] = _wrap_idx16(hi)

    snode = np.ones((n_cores, P, n_windows), np.float32)
    snode[node_core, node_slot, node_window] = s_all.astype(np.float32)

    # host-side inverse permutation: node v lives at core_out row
    # node_window*128 + node_slot of core node_core
    out_core = node_core
    out_row = node_window * P + node_slot

    return (
        idx16, sel, snode, out_deg, out_core, out_row,
        n_windows, T_lo, T_hi, n_cols_sel, ntl, nth, off,
    )


def _build_nc(N, D, n_windows, T_lo, T_hi, n_cols_sel, ntl, nth, off):
    import concourse.bacc as bacc
    import concourse.tile as tile
    from concourse import mybir

    f32 = mybir.dt.float32
    bf16 = mybir.dt.bfloat16
    fp8 = mybir.dt.float8e4
    i16 = mybir.dt.int16
    T = T_lo + T_hi
    CL, CH = T_lo * 8, T_hi * 8
    CLa = (CL + 31) // 32 * 32
    CHa = (CH + 31) // 32 * 32

    nc = bacc.Bacc(
        None, target_bir_lowering=False, debug=False, num_swdge_queues=N_QUEUES
    )
    h_d = nc.declare_dram_parameter("h16", [N, D], bf16, isOutput=False)
    idx_d = nc.declare_dram_parameter(
        "idx16", [P, n_windows * (CLa + CHa)], i16, isOutput=False
    )
    sel_d = nc.declare_dram_parameter(
        "sel8", [P, n_cols_sel * P], fp8, isOutput=False
    )
    w_d = nc.declare_dram_parameter("w16", [P, D], bf16, isOutput=False)
    fc_d = nc.declare_dram_parameter(
        "fconst", [P, D + n_windows], f32, isOutput=False
    )
    out_d = nc.declare_dram_parameter("out", [n_windows * P, D], bf16, isOutput=True)

    mult = mybir.AluOpType.mult

    with tile.TileContext(nc) as tc:
        with (
            tc.tile_pool(name="singles", bufs=1) as singles,
            tc.tile_pool(name="glo", bufs=12) as glopool,
            tc.tile_pool(name="ghi", bufs=12) as ghipool,
            tc.tile_pool(name="sel", bufs=8) as selpool,
            tc.tile_pool(name="psum", bufs=6, space="PSUM") as psumpool,
            tc.tile_pool(name="psum2", bufs=2, space="PSUM") as psum2pool,
            tc.tile_pool(name="outp", bufs=3) as outpool,
        ):
            idx_sb = singles.tile([P, n_windows * (CLa + CHa)], i16)
            tot = n_windows * (CLa + CHa)
            hd = min(2, n_windows) * (CLa + CHa)
            nc.sync.dma_start(out=idx_sb[:, :hd], in_=idx_d[:, :hd])
            if hd < tot:
                nc.sync.dma_start(out=idx_sb[:, hd:], in_=idx_d[:, hd:])
            w_sb = singles.tile([P, D], bf16)
            nc.sync.dma_start(out=w_sb[:], in_=w_d[:])
            fc_sb = singles.tile([P, D + n_windows], f32)
            nc.sync.dma_start(out=fc_sb[:], in_=fc_d[:])

            b_sb = fc_sb[:, 0:D]
            s_sb = fc_sb[:, D : D + n_windows]

            agg = singles.tile([P, n_windows * P], bf16)  # agg^T [feat, node]

            def _phase2(w2):
                ps2 = psum2pool.tile([P, D], f32)
                nc.tensor.matmul(
                    out=ps2[:],
                    lhsT=agg[:, w2 * P : (w2 + 1) * P],
                    rhs=w_sb,
                    start=True,
                    stop=True,
                )
                o = outpool.tile([P, D], bf16)
                ot = outpool.tile([P, D], f32, tag="ot")
                nc.vector.tensor_tensor(
                    out=ot[:],
                    in0=ps2[:],
                    in1=s_sb[:, w2 : w2 + 1].to_broadcast([P, D]),
                    op=mult,
                )
                nc.vector.tensor_add(out=o[:], in0=ot[:], in1=b_sb)
                nc.sync.dma_start(out=out_d[w2 * P : (w2 + 1) * P, :], in_=o[:])

            h_lo = h_d[0 : min(SPLIT, N), :]
            hi_base = SPLIT if N > SPLIT else 0
            h_hi = h_d[hi_base:N, :]

            qctr = 0
            for w in range(n_windows):
                base = w * (CLa + CHa)
                ntl_w, nth_w = int(ntl[w]), int(nth[w])
                nlv_w = ntl_w + nth_w

                def _even_chunks(n_tiles):
                    n_ck = (n_tiles + GCH - 1) // GCH
                    bsz, rem = divmod(n_tiles, n_ck)
                    return [bsz + (1 if k < rem else 0) for k in range(n_ck)]

                # even chunk sizes (e.g. 11 -> 6+5, not 8+3): uniform gather
                # durations pack the 4 queues / 16 engines more smoothly
                lo_tiles = []
                start = 0
                for sz in _even_chunks(ntl_w):
                    g = glopool.tile([P, GCH, P], bf16, tag="glo")
                    cb = base + start * 8
                    nc.gpsimd.dma_gather(
                        g[:, :sz, :],
                        h_lo,
                        idx_sb[:, cb : cb + sz * 8],
                        sz * P,
                        sz * P,
                        P,
                        single_packet=True,
                        queue_num=qctr % N_QUEUES,
                    )
                    qctr += 1
                    lo_tiles += [(g, t) for t in range(sz)]
                    start += sz
                hi_tiles = []
                start = 0
                for sz in _even_chunks(nth_w):
                    g = ghipool.tile([P, GCH, P], bf16, tag="ghi")
                    cb = base + CLa + start * 8
                    nc.gpsimd.dma_gather(
                        g[:, :sz, :],
                        h_hi,
                        idx_sb[:, cb : cb + sz * 8],
                        sz * P,
                        sz * P,
                        P,
                        single_packet=True,
                        queue_num=qctr % N_QUEUES,
                    )
                    qctr += 1
                    hi_tiles += [(g, t) for t in range(sz)]
                    start += sz
                sel_sb = selpool.tile([P, T * P], fp8)
                nc.sync.dma_start(
                    out=sel_sb[:, : nlv_w * P],
                    in_=sel_d[:, int(off[w]) * P : int(off[w + 1]) * P],
                )
                ps = psumpool.tile([P, P], f32)
                for i in range(nlv_w):
                    if i < ntl_w:
                        g, t = lo_tiles[i]
                    else:
                        g, t = hi_tiles[i - ntl_w]
                    lhsT = g[:, t, :]
                    nc.tensor.matmul(
                        out=ps[:],
                        lhsT=lhsT,
                        rhs=sel_sb[:, i * P : (i + 1) * P],
                        start=(i == 0),
                        stop=(i == nlv_w - 1),
                    )
                nc.scalar.copy(out=agg[:, w * P : (w + 1) * P], in_=ps[:])
                # phase 2 inline: the window's output work hides in the
                # gather shadow of subsequent windows
                _phase2(w)

    nc.compile()
    return nc


def kernel(h, src, dst, distance, weight, bias, _trace=False):
    import ml_dtypes

    from concourse.bass_utils import run_bass_kernel_spmd

    bf16 = ml_dtypes.bfloat16
    fp8 = ml_dtypes.float8_e4m3

    h = np.ascontiguousarray(np.asarray(h, dtype=np.float32))
    weight = np.asarray(weight, dtype=np.float32)
    bias = np.asarray(bias, dtype=np.float32)
    N, D = h.shape

    (
        idx16, sel, snode, out_deg, out_core, out_row,
        n_windows, T_lo, T_hi, n_cols_sel, ntl, nth, off,
    ) = _prep_host(h, src, dst, distance, N_CORES)

    # gathered table: source-side normalized features, bf16 rows (256B)
    feat16 = np.ascontiguousarray(
        (h * (out_deg**-0.5)[:, None].astype(np.float32)).astype(bf16)
    )
    w16 = np.ascontiguousarray(weight.astype(bf16))
    biasf = np.broadcast_to(bias[None, :], (P, D))

    nc = _build_nc(N, D, n_windows, T_lo, T_hi, n_cols_sel, ntl, nth, off)

    in_maps = []
    for c in range(N_CORES):
        fconst = np.ascontiguousarray(
            np.concatenate([biasf, snode[c]], axis=1).astype(np.float32)
        )
        in_maps.append(
            {
                "h16": feat16,
                "idx16": np.ascontiguousarray(idx16[c]),
                "sel8": np.ascontiguousarray(sel[c].astype(fp8)),
                "w16": w16,
                "fconst": fconst,
            }
        )

    import os

    _tmpdir = os.environ.get("BASS_TMPDIR") or None
    res = run_bass_kernel_spmd(
        nc, in_maps, list(range(N_CORES)), trace=_trace, tmpdir=_tmpdir
    )

    stacked = np.stack(
        [np.asarray(res.results[c]["out"]).astype(np.float32) for c in range(N_CORES)]
    )
    out = stacked[out_core, out_row].astype(np.float32)

    if _trace:
        return out, res
    return out

